# revision 3
# baseline (speedup 1.0000x reference)
"""Trainium2 Bass kernel for nn_Attention_48206712930624.

Dense transformer block: LayerNorm -> QKV proj -> 8-head attention
(head_dim = 512) -> output projection.  B=4, S=2048, D=512, H=8.

Sharding: tensor-parallel over heads -- each of the 8 NeuronCores computes
one head end-to-end (LN duplicated), producing a partial output projection
Y_h = (P_h @ V_h) @ o_w_h (un-normalized) plus the softmax denominators
l_h.  The host combines:  out = sum_h Y_h / l_h + const.

Device-side design notes:
  * All big matmuls run in float32r (TF32-like, full PE rate at N=512,
    ~1.5e-4 rel err) with fp32 PSUM accumulation.
  * Scores are computed TRANSPOSED (S^T[k,q] = k^T.T @ q^T) so softmax's
    exp is orientation-agnostic (ACT elementwise) and P^T lands directly
    in the layout att@V needs (k on partitions).  Row sums l are computed
    with a ones-vector matmul.  No max-subtraction (logits ~ N(0,1);
    folded scale keeps exp well within fp32 range).
  * LayerNorm scale/bias and the attention scale are folded into the
    weights on the host; v-bias and o_b fold into a constant row added on
    the host.  rstd = exp(-0.5*ln(var+eps)) keeps the whole kernel on one
    ACT table set (natural_log_exp_and_others).
"""

import sys

import numpy as np

for _p in ("/opt/trn_rl_repo", "/root/.axon_site/_ro/trn_rl_repo"):
    if _p not in sys.path:
        sys.path.append(_p)

import concourse.bacc as bacc
import concourse.mybir as mybir
import concourse.tile as tile
from concourse.bass_utils import run_bass_kernel_spmd
from concourse.masks import make_identity

B, S, D, H = 4, 2048, 512, 8
P = 128
DC = D // P          # head/model dim chunks (4)
KC = S // P          # k chunks per batch (16)
QB = 512             # q-block size
NQB = S // QB        # q blocks per batch (4)
EPS = 1e-5
F32 = mybir.dt.float32
F32R = mybir.dt.float32r
AF = mybir.ActivationFunctionType
ALU = mybir.AluOpType

N_CORES = 8

_CACHE = {}


def build():
    nc = bacc.Bacc("TRN2", target_bir_lowering=False, debug=False,
                   num_devices=N_CORES)
    x = nc.dram_tensor("x", [B, S, D], F32, kind="ExternalInput").ap()
    w_drams = {
        n: nc.dram_tensor(n, [D, D], F32, kind="ExternalInput").ap()
        for n in ("qw", "kw", "vw", "ow")
    }
    qb_d = nc.dram_tensor("qb", [D], F32, kind="ExternalInput").ap()
    kb_d = nc.dram_tensor("kb", [D], F32, kind="ExternalInput").ap()
    y = nc.dram_tensor("y", [B, S, D], F32, kind="ExternalOutput").ap()
    lsum = nc.dram_tensor("lsum", [B, S], F32, kind="ExternalOutput").ap()

    with tile.TileContext(nc) as tc:
        with (
            tc.tile_pool(name="const", bufs=1) as const,
            tc.tile_pool(name="wts", bufs=1) as wts,
            tc.tile_pool(name="kv", bufs=1) as kv,
            tc.tile_pool(name="big", bufs=2) as big,
            tc.tile_pool(name="qt", bufs=2) as qtp,
            tc.tile_pool(name="ot", bufs=1) as otp,
            tc.tile_pool(name="stage", bufs=3) as stage,
            tc.tile_pool(name="stats", bufs=4) as stats,
            tc.tile_pool(name="lsb", bufs=2) as lsbp,
            tc.tile_pool(name="psum", bufs=1, space="PSUM") as psum,
        ):
            # ---- constants ----
            ident = const.tile([P, P], F32)
            make_identity(nc, ident)
            ones_raw = const.tile([P, 1], F32)
            nc.vector.memset(ones_raw, 1.0)
            ones_r = const.tile([P, 1], F32R)
            nc.vector.tensor_copy(ones_r, ones_raw.bitcast(F32R))
            eps_t = const.tile([P, 1], F32)
            nc.vector.memset(eps_t, EPS)
            qb_t = const.tile([P, DC], F32)
            nc.sync.dma_start(out=qb_t, in_=qb_d.rearrange("(c p) -> p c", p=P))
            kb_t = const.tile([P, DC], F32)
            nc.sync.dma_start(out=kb_t, in_=kb_d.rearrange("(c p) -> p c", p=P))

            # ---- weights: load + round to f32r ----
            w_r = {}
            for n, dram in w_drams.items():
                wst = big.tile([P, DC, D], F32, name=f"{n}_stage", tag="big")
                nc.sync.dma_start(out=wst,
                                  in_=dram.rearrange("(c p) n -> p c n", p=P))
                w_r[n] = wts.tile([P, DC, D], F32R, name=f"{n}_r", tag=n)
                nc.vector.tensor_copy(w_r[n], wst.bitcast(F32R))

            for b in range(B):
                # ---- phase A: LayerNorm + transpose -> xhT [d, r] ----
                xhT = big.tile([P, DC, S], F32R, name=f"xhT{b}", tag="big")
                for rt in range(KC):
                    xt = stage.tile([P, D], F32, name="xt", tag="xt")
                    nc.sync.dma_start(out=xt, in_=x[b, rt * P:(rt + 1) * P, :])
                    st6 = stats.tile([P, 6], F32, name="st6", tag="st6")
                    nc.vector.bn_stats(out=st6, in_=xt)
                    mv = stats.tile([P, 2], F32, name="mv", tag="mv")
                    nc.vector.bn_aggr(out=mv, in_=st6)
                    # rstd = exp(-0.5 * ln(var + eps))
                    lnv = stats.tile([P, 1], F32, name="lnv", tag="lnv")
                    nc.scalar.activation(out=lnv, in_=mv[:, 1:2], func=AF.Ln,
                                         bias=eps_t)
                    rstd = stats.tile([P, 1], F32, name="rstd", tag="rstd")
                    nc.scalar.activation(out=rstd, in_=lnv, func=AF.Exp,
                                         scale=-0.5)
                    xh = stage.tile([P, D], F32, name="xh", tag="xh")
                    nc.vector.tensor_scalar(out=xh, in0=xt,
                                            scalar1=mv[:, 0:1], scalar2=rstd,
                                            op0=ALU.subtract, op1=ALU.mult)
                    tp = psum.tile([P, D], F32, name="tp", tag="s", bufs=2)
                    for dc in range(DC):
                        nc.tensor.transpose(tp[:, dc * P:(dc + 1) * P],
                                            xh[:, dc * P:(dc + 1) * P], ident)
                    nc.vector.tensor_copy(
                        out=xhT[:, :, rt * P:(rt + 1) * P],
                        in_=tp.rearrange("p (c r) -> p c r", c=DC).bitcast(F32R))

                # ---- phase B: k^T and v projections (full batch) ----
                kT = kv.tile([P, DC, S], F32R, name=f"kT{b}", tag="kT")
                for cc in range(DC):
                    kps = psum.tile([P, NQB, QB], F32, name="kps", tag="o",
                                    bufs=1)
                    for dc in range(DC):
                        for nb in range(NQB):
                            nc.tensor.matmul(
                                kps[:, nb, :],
                                w_r["kw"][:, dc, cc * P:(cc + 1) * P],
                                xhT[:, dc, nb * QB:(nb + 1) * QB],
                                start=(dc == 0), stop=(dc == DC - 1))
                    nc.scalar.add(out=kT[:, cc, :],
                                  in_=kps.rearrange("p a b -> p (a b)")
                                  .bitcast(F32R),
                                  add=kb_t[:, cc:cc + 1])
                v_t = kv.tile([P, KC, D], F32R, name=f"v{b}", tag="v")
                for rc in range(KC):
                    vps = psum.tile([P, D], F32, name="vps", tag="s", bufs=2)
                    for dc in range(DC):
                        nc.tensor.matmul(vps,
                                         xhT[:, dc, rc * P:(rc + 1) * P],
                                         w_r["vw"][:, dc, :],
                                         start=(dc == 0), stop=(dc == DC - 1))
                    nc.vector.tensor_copy(out=v_t[:, rc, :],
                                          in_=vps.bitcast(F32R))

                # ---- phase C: attention + output projection per q-block ----
                for qb_i in range(NQB):
                    q0 = qb_i * QB
                    # q^T for this block
                    qT = qtp.tile([P, DC, QB], F32R, name="qT", tag="qT")
                    for cc in range(DC):
                        qps = psum.tile([P, QB], F32, name="qps", tag="s",
                                        bufs=2)
                        for dc in range(DC):
                            nc.tensor.matmul(
                                qps, w_r["qw"][:, dc, cc * P:(cc + 1) * P],
                                xhT[:, dc, q0:q0 + QB],
                                start=(dc == 0), stop=(dc == DC - 1))
                        nc.scalar.add(out=qT[:, cc, :],
                                      in_=qps.bitcast(F32R),
                                      add=qb_t[:, cc:cc + 1])

                    pT = big.tile([P, KC, QB], F32R, name="pT", tag="big")
                    o_ps = psum.tile([P, DC, QB], F32, name="o_ps", tag="o",
                                     bufs=1)
                    l_ps = psum.tile([1, QB], F32, name="l_ps", tag="ly",
                                     bufs=2)
                    # software-pipelined: scores/exp for kc while att@V for kc-1
                    for kc in range(KC + 1):
                        if kc < KC:
                            sps = psum.tile([P, QB], F32, name="sps", tag="s",
                                            bufs=2)
                            for dc in range(DC):
                                nc.tensor.matmul(
                                    sps, kT[:, dc, kc * P:(kc + 1) * P],
                                    qT[:, dc, :],
                                    start=(dc == 0), stop=(dc == DC - 1))
                            nc.scalar.activation(out=pT[:, kc, :],
                                                 in_=sps.bitcast(F32R),
                                                 func=AF.Exp)
                        if kc >= 1:
                            k2 = kc - 1
                            nc.tensor.matmul(l_ps, ones_r, pT[:, k2, :],
                                             start=(k2 == 0),
                                             stop=(k2 == KC - 1))
                            for dc in range(DC):
                                nc.tensor.matmul(
                                    o_ps[:, dc, :],
                                    v_t[:, k2, dc * P:(dc + 1) * P],
                                    pT[:, k2, :],
                                    start=(k2 == 0), stop=(k2 == KC - 1))

                    l_sb = lsbp.tile([1, QB], F32, name="l_sb", tag="l")
                    nc.vector.tensor_copy(out=l_sb, in_=l_ps)
                    nc.sync.dma_start(out=lsum[b, q0:q0 + QB].unsqueeze(0),
                                      in_=l_sb)

                    oT = otp.tile([P, DC, QB], F32R, name="oT", tag="oT")
                    nc.vector.tensor_copy(out=oT, in_=o_ps.bitcast(F32R))
                    for qc in range(QB // P):
                        yps = psum.tile([P, D], F32, name="yps", tag="ly",
                                        bufs=2)
                        for dc in range(DC):
                            nc.tensor.matmul(
                                yps, oT[:, dc, qc * P:(qc + 1) * P],
                                w_r["ow"][:, dc, :],
                                start=(dc == 0), stop=(dc == DC - 1))
                        yt = stage.tile([P, D], F32, name="yt", tag="yt")
                        nc.vector.tensor_copy(out=yt, in_=yps)
                        r0 = q0 + qc * P
                        nc.sync.dma_start(out=y[b, r0:r0 + P, :], in_=yt)

    nc.compile()
    return nc


def _prep_core_inputs(inputs, h):
    """Fold LN affine + attention scale into per-head weights (float64)."""
    x = np.asarray(inputs["x"], np.float32)
    ln_w = np.asarray(inputs["ln_w"], np.float64)
    ln_b = np.asarray(inputs["ln_b"], np.float64)
    sl = slice(h * D, (h + 1) * D)
    scale = float(D) ** -0.5
    q_w = np.asarray(inputs["q_w"], np.float64)[:, sl]
    k_w = np.asarray(inputs["k_w"], np.float64)[:, sl]
    v_w = np.asarray(inputs["v_w"], np.float64)[:, sl]
    o_w = np.asarray(inputs["o_w"], np.float64)[sl, :]
    q_b = np.asarray(inputs["q_b"], np.float64)[sl]
    k_b = np.asarray(inputs["k_b"], np.float64)[sl]
    qw = (ln_w[:, None] * q_w) * scale
    kw = ln_w[:, None] * k_w
    vw = ln_w[:, None] * v_w
    qb = (ln_b @ q_w + q_b) * scale
    kb = ln_b @ k_w + k_b
    return {
        "x": x,
        "qw": qw.astype(np.float32), "kw": kw.astype(np.float32),
        "vw": vw.astype(np.float32), "ow": o_w.astype(np.float32),
        "qb": qb.astype(np.float32), "kb": kb.astype(np.float32),
    }


def kernel(**inputs):
    if "nc" not in _CACHE:
        _CACHE["nc"] = build()
    nc = _CACHE["nc"]

    in_maps = [_prep_core_inputs(inputs, h) for h in range(N_CORES)]
    res = run_bass_kernel_spmd(nc, in_maps, core_ids=list(range(N_CORES)))

    out = np.zeros((B, S, D), np.float64)
    for h in range(N_CORES):
        yh = res.results[h]["y"].astype(np.float64)
        lh = res.results[h]["lsum"].astype(np.float64)
        out += yh / lh[..., None]

    # host-folded constant row: sum_h vb_h @ ow_h + o_b
    ln_b = np.asarray(inputs["ln_b"], np.float64)
    v_w = np.asarray(inputs["v_w"], np.float64)
    v_b = np.asarray(inputs["v_b"], np.float64)
    o_w = np.asarray(inputs["o_w"], np.float64)
    o_b = np.asarray(inputs["o_b"], np.float64)
    vb_full = ln_b @ v_w + v_b            # [D*H]
    out += vb_full @ o_w + o_b
    return out.astype(np.float32)


# revision 5
# speedup vs baseline: 1.0877x; 1.0877x over previous
"""Trainium2 Bass kernel for nn_Attention_48206712930624.

Dense transformer block: LayerNorm -> QKV proj -> 8-head attention
(head_dim = 512) -> output projection.  B=4, S=2048, D=512, H=8.

Sharding: tensor-parallel over heads -- each of the 8 NeuronCores computes
one head end-to-end (LN duplicated), producing a partial output projection
Y_h = (P_h @ V_h) @ o_w_h (un-normalized) plus the softmax denominators
l_h.  The host combines:  out = sum_h Y_h / l_h + const.

Device-side design notes:
  * All big matmuls run in float32r (TF32-like, full PE rate at N=512,
    ~1.5e-4 rel err) with fp32 PSUM accumulation.
  * Scores are computed TRANSPOSED (S^T[k,q] = k^T.T @ q^T) so softmax's
    exp is orientation-agnostic (ACT elementwise) and P^T lands directly
    in the layout att@V needs (k on partitions).  Row sums l are computed
    with a ones-vector matmul.  No max-subtraction (logits ~ N(0,1);
    folded scale keeps exp well within fp32 range).
  * LayerNorm scale/bias and the attention scale are folded into the
    weights on the host; v-bias and o_b fold into a constant row added on
    the host.  rstd = exp(-0.5*ln(var+eps)) and DVE-side bias adds keep
    the whole kernel on ONE ACT table set (natural_log_exp_and_others).
  * Scores use paired PSUM banks so each exp covers [128,1024].
"""

import sys

import numpy as np

for _p in ("/opt/trn_rl_repo", "/root/.axon_site/_ro/trn_rl_repo"):
    if _p not in sys.path:
        sys.path.append(_p)

import concourse.bacc as bacc
import concourse.mybir as mybir
import concourse.tile as tile
from concourse.bass_utils import run_bass_kernel_spmd
from concourse.masks import make_identity

# Steer the ACT-table-load placement pass to the one set that holds every
# function this kernel uses (ln, exp, copy, identity), so the whole kernel
# runs on a single table load instead of thrashing between per-function
# sets.  Only the pass's view is doctored; runtime tables are untouched.
_ONE_SET = "natural_log_exp_and_others"
_orig_get_act_tables = bacc.get_activation_tables


def _patched_get_act_tables(arch):
    t = _orig_get_act_tables(arch)
    af = mybir.ActivationFunctionType
    strip = {af.Ln, af.Exp, af.Copy, af.Identity}
    return {
        name: (set(fns) if name == _ONE_SET else set(fns) - strip)
        for name, fns in t.items()
    }


bacc.get_activation_tables = _patched_get_act_tables

B, S, D, H = 4, 2048, 512, 8
P = 128
DC = D // P          # head/model dim chunks (4)
KC = S // P          # k chunks per batch (16)
QB = 512             # q-block size
NQB = S // QB        # q blocks per batch (4)
EPS = 1e-5
F32 = mybir.dt.float32
F32R = mybir.dt.float32r
AF = mybir.ActivationFunctionType
ALU = mybir.AluOpType

N_CORES = 8

_CACHE = {}


def build():
    nc = bacc.Bacc("TRN2", target_bir_lowering=False, debug=False,
                   num_devices=N_CORES)
    x = nc.dram_tensor("x", [B, S, D], F32, kind="ExternalInput").ap()
    w_drams = {
        n: nc.dram_tensor(n, [D, D], F32, kind="ExternalInput").ap()
        for n in ("qw", "kw", "vw", "ow")
    }
    qb_d = nc.dram_tensor("qb", [D], F32, kind="ExternalInput").ap()
    kb_d = nc.dram_tensor("kb", [D], F32, kind="ExternalInput").ap()
    y = nc.dram_tensor("y", [B, S, D], F32, kind="ExternalOutput").ap()
    lsum = nc.dram_tensor("lsum", [B, S], F32, kind="ExternalOutput").ap()

    with tile.TileContext(nc) as tc:
        with (
            tc.tile_pool(name="const", bufs=1) as const,
            tc.tile_pool(name="wts", bufs=1) as wts,
            tc.tile_pool(name="kv", bufs=1) as kv,
            tc.tile_pool(name="big", bufs=2) as big,
            tc.tile_pool(name="qt", bufs=1) as qtp,
            tc.tile_pool(name="ot", bufs=1) as otp,
            tc.tile_pool(name="stage", bufs=1) as stage,
            tc.tile_pool(name="stats", bufs=4) as stats,
            tc.tile_pool(name="lsb", bufs=1) as lsbp,
            tc.tile_pool(name="psum", bufs=1, space="PSUM") as psum,
        ):
            # ---- constants ----
            ident = const.tile([P, P], F32)
            make_identity(nc, ident)
            ones_raw = const.tile([P, 1], F32)
            nc.vector.memset(ones_raw, 1.0)
            ones_r = const.tile([P, 1], F32R)
            nc.vector.tensor_copy(ones_r, ones_raw.bitcast(F32R))
            eps_t = const.tile([P, 1], F32)
            nc.vector.memset(eps_t, EPS)
            qb_t = const.tile([P, DC], F32)
            nc.sync.dma_start(out=qb_t, in_=qb_d.rearrange("(c p) -> p c", p=P))
            kb_t = const.tile([P, DC], F32)
            nc.sync.dma_start(out=kb_t, in_=kb_d.rearrange("(c p) -> p c", p=P))

            # ---- weights: load + round to f32r ----
            w_r = {}
            for n, dram in w_drams.items():
                wst = big.tile([P, DC, D], F32, name=f"{n}_stage", tag="big")
                nc.sync.dma_start(out=wst,
                                  in_=dram.rearrange("(c p) n -> p c n", p=P))
                w_r[n] = wts.tile([P, DC, D], F32R, name=f"{n}_r", tag=n)
                nc.vector.tensor_copy(w_r[n], wst.bitcast(F32R))

            for b in range(B):
                # ---- phase A: LayerNorm + transpose -> xhT [d, r] ----
                xhT = big.tile([P, DC, S], F32R, name=f"xhT{b}", tag="big")
                for g in range(KC // 4):
                    xg = stage.tile([P, 4, D], F32, name="xg", tag="xg", bufs=2)
                    r0 = g * 4 * P
                    nc.sync.dma_start(
                        out=xg,
                        in_=x[b, r0:r0 + 4 * P, :]
                        .rearrange("(j p) d -> p j d", p=P))
                    for j in range(4):
                        rt = g * 4 + j
                        xt = xg[:, j, :]
                        st6 = stats.tile([P, 6], F32, name="st6", tag="st6")
                        nc.vector.bn_stats(out=st6, in_=xt)
                        mv = stats.tile([P, 2], F32, name="mv", tag="mv")
                        nc.vector.bn_aggr(out=mv, in_=st6)
                        # rstd = exp(-0.5 * ln(var + eps))
                        lnv = stats.tile([P, 1], F32, name="lnv", tag="lnv")
                        nc.scalar.activation(out=lnv, in_=mv[:, 1:2],
                                             func=AF.Ln, bias=eps_t)
                        rstd = stats.tile([P, 1], F32, name="rstd", tag="rstd")
                        nc.scalar.activation(out=rstd, in_=lnv, func=AF.Exp,
                                             scale=-0.5)
                        xh = stage.tile([P, D], F32, name="xh", tag="xh",
                                        bufs=2)
                        nc.vector.tensor_scalar(out=xh, in0=xt,
                                                scalar1=mv[:, 0:1],
                                                scalar2=rstd,
                                                op0=ALU.subtract, op1=ALU.mult)
                        tp = psum.tile([P, D], F32, name="tp", tag="s", bufs=2)
                        for dc in range(DC):
                            nc.tensor.transpose(tp[:, dc * P:(dc + 1) * P],
                                                xh[:, dc * P:(dc + 1) * P],
                                                ident)
                        nc.vector.tensor_copy(
                            out=xhT[:, :, rt * P:(rt + 1) * P],
                            in_=tp.rearrange("p (c r) -> p c r", c=DC)
                            .bitcast(F32R))

                # ---- phase B: k^T and v projections (full batch) ----
                kT = kv.tile([P, DC, S], F32R, name=f"kT{b}", tag="kT")
                for cc in range(DC):
                    for hf in range(2):
                        kps = psum.tile([P, 2, QB], F32, name="kps", tag="s",
                                        bufs=2)
                        for dc in range(DC):
                            for j in range(2):
                                q0 = (hf * 2 + j) * QB
                                nc.tensor.matmul(
                                    kps[:, j, :],
                                    w_r["kw"][:, dc, cc * P:(cc + 1) * P],
                                    xhT[:, dc, q0:q0 + QB],
                                    start=(dc == 0), stop=(dc == DC - 1))
                        nc.vector.tensor_scalar_add(
                            out=kT[:, cc, hf * 2 * QB:(hf + 1) * 2 * QB],
                            in0=kps.rearrange("p j q -> p (j q)").bitcast(F32R),
                            scalar1=kb_t[:, cc:cc + 1])
                v_t = kv.tile([P, KC, D], F32R, name=f"v{b}", tag="v")
                for rp in range(KC // 2):
                    vps = psum.tile([P, 2, D], F32, name="vps", tag="s",
                                    bufs=2)
                    for dc in range(DC):
                        for j in range(2):
                            rc = rp * 2 + j
                            nc.tensor.matmul(
                                vps[:, j, :],
                                xhT[:, dc, rc * P:(rc + 1) * P],
                                w_r["vw"][:, dc, :],
                                start=(dc == 0), stop=(dc == DC - 1))
                    nc.vector.tensor_copy(out=v_t[:, rp * 2:rp * 2 + 2, :],
                                          in_=vps.bitcast(F32R))

                # ---- phase C: attention + output projection per q-block ----
                for qb_i in range(NQB):
                    q0 = qb_i * QB
                    # q^T for this block
                    qT = qtp.tile([P, DC, QB], F32R, name="qT", tag="qT")
                    for cp in range(DC // 2):
                        qps = psum.tile([P, 2, QB], F32, name="qps", tag="s",
                                        bufs=2)
                        for dc in range(DC):
                            for j in range(2):
                                cc = cp * 2 + j
                                nc.tensor.matmul(
                                    qps[:, j, :],
                                    w_r["qw"][:, dc, cc * P:(cc + 1) * P],
                                    xhT[:, dc, q0:q0 + QB],
                                    start=(dc == 0), stop=(dc == DC - 1))
                        for j in range(2):
                            cc = cp * 2 + j
                            nc.vector.tensor_scalar_add(
                                out=qT[:, cc, :],
                                in0=qps[:, j, :].bitcast(F32R),
                                scalar1=qb_t[:, cc:cc + 1])

                    pT = big.tile([P, KC, QB], F32R, name="pT", tag="big")
                    # scores + exp, two k-chunks per PSUM pair
                    for kp in range(KC // 2):
                        sps = psum.tile([P, 2, QB], F32, name="sps", tag="s",
                                        bufs=2)
                        for dc in range(DC):
                            for j in range(2):
                                kc = kp * 2 + j
                                nc.tensor.matmul(
                                    sps[:, j, :],
                                    kT[:, dc, kc * P:(kc + 1) * P],
                                    qT[:, dc, :],
                                    start=(dc == 0), stop=(dc == DC - 1))
                        nc.scalar.activation(out=pT[:, kp * 2:kp * 2 + 2, :],
                                             in_=sps.bitcast(F32R),
                                             func=AF.Exp)
                    # softmax denominators
                    l_ps = psum.tile([1, QB], F32, name="l_ps", tag="ly",
                                     bufs=2)
                    for kc in range(KC):
                        nc.tensor.matmul(l_ps, ones_r, pT[:, kc, :],
                                         start=(kc == 0), stop=(kc == KC - 1))
                    l_sb = lsbp.tile([1, QB], F32, name="l_sb", tag="l")
                    nc.vector.tensor_copy(out=l_sb, in_=l_ps)
                    nc.sync.dma_start(out=lsum[b, q0:q0 + QB].unsqueeze(0),
                                      in_=l_sb)
                    # att @ V, one d-chunk at a time
                    oT = otp.tile([P, DC, QB], F32R, name="oT", tag="oT")
                    for dc in range(DC):
                        o_ps = psum.tile([P, QB], F32, name="o_ps", tag="o",
                                         bufs=2)
                        for kc in range(KC):
                            nc.tensor.matmul(
                                o_ps, v_t[:, kc, dc * P:(dc + 1) * P],
                                pT[:, kc, :],
                                start=(kc == 0), stop=(kc == KC - 1))
                        nc.scalar.copy(out=oT[:, dc, :],
                                       in_=o_ps.bitcast(F32R))
                    # output projection
                    for qc in range(QB // P):
                        yps = psum.tile([P, D], F32, name="yps", tag="ly",
                                        bufs=2)
                        for dc in range(DC):
                            nc.tensor.matmul(
                                yps, oT[:, dc, qc * P:(qc + 1) * P],
                                w_r["ow"][:, dc, :],
                                start=(dc == 0), stop=(dc == DC - 1))
                        yt = stage.tile([P, D], F32, name="yt", tag="yt",
                                        bufs=3)
                        nc.vector.tensor_copy(out=yt, in_=yps)
                        r0 = q0 + qc * P
                        nc.sync.dma_start(out=y[b, r0:r0 + P, :], in_=yt)

    nc.compile()
    return nc


def _prep_core_inputs(inputs, h):
    """Fold LN affine + attention scale into per-head weights (float64)."""
    x = np.asarray(inputs["x"], np.float32)
    ln_w = np.asarray(inputs["ln_w"], np.float64)
    ln_b = np.asarray(inputs["ln_b"], np.float64)
    sl = slice(h * D, (h + 1) * D)
    scale = float(D) ** -0.5
    q_w = np.asarray(inputs["q_w"], np.float64)[:, sl]
    k_w = np.asarray(inputs["k_w"], np.float64)[:, sl]
    v_w = np.asarray(inputs["v_w"], np.float64)[:, sl]
    o_w = np.asarray(inputs["o_w"], np.float64)[sl, :]
    q_b = np.asarray(inputs["q_b"], np.float64)[sl]
    k_b = np.asarray(inputs["k_b"], np.float64)[sl]
    qw = (ln_w[:, None] * q_w) * scale
    kw = ln_w[:, None] * k_w
    vw = ln_w[:, None] * v_w
    qb = (ln_b @ q_w + q_b) * scale
    kb = ln_b @ k_w + k_b
    return {
        "x": x,
        "qw": qw.astype(np.float32), "kw": kw.astype(np.float32),
        "vw": vw.astype(np.float32), "ow": o_w.astype(np.float32),
        "qb": qb.astype(np.float32), "kb": kb.astype(np.float32),
    }


def kernel(**inputs):
    if "nc" not in _CACHE:
        _CACHE["nc"] = build()
    nc = _CACHE["nc"]

    in_maps = [_prep_core_inputs(inputs, h) for h in range(N_CORES)]
    res = run_bass_kernel_spmd(nc, in_maps, core_ids=list(range(N_CORES)))

    out = np.zeros((B, S, D), np.float64)
    for h in range(N_CORES):
        yh = res.results[h]["y"].astype(np.float64)
        lh = res.results[h]["lsum"].astype(np.float64)
        out += yh / lh[..., None]

    # host-folded constant row: sum_h vb_h @ ow_h + o_b
    ln_b = np.asarray(inputs["ln_b"], np.float64)
    v_w = np.asarray(inputs["v_w"], np.float64)
    v_b = np.asarray(inputs["v_b"], np.float64)
    o_w = np.asarray(inputs["o_w"], np.float64)
    o_b = np.asarray(inputs["o_b"], np.float64)
    vb_full = ln_b @ v_w + v_b            # [D*H]
    out += vb_full @ o_w + o_b
    return out.astype(np.float32)


# revision 20
# speedup vs baseline: 12218.5217x; 11233.5844x over previous
"""Trainium2 Bass kernel for nn_Attention_48206712930624.

Dense transformer block: LayerNorm -> QKV proj -> 8-head attention
(head_dim = 512) -> output projection.  B=4, S=2048, D=512, H=8.

Sharding: tensor-parallel over heads -- each of the 8 NeuronCores computes
one head end-to-end (LN duplicated), producing a partial output projection
Y_h = (P_h @ V_h) @ o_w_h (un-normalized) plus the softmax denominators
l_h.  The host combines:  out = sum_h Y_h / l_h + const.

Device-side design notes:
  * All big matmuls run in float32r (TF32-like, full PE rate at N=512,
    ~1.5e-4 rel err) with fp32 PSUM accumulation.
  * Scores are computed TRANSPOSED (S^T[k,q] = k^T.T @ q^T) so softmax's
    exp is orientation-agnostic (ACT elementwise) and P^T lands directly
    in the layout att@V needs (k on partitions).  Row sums l are computed
    with a ones-vector matmul.  No max-subtraction (logits ~ N(0,1);
    folded scale keeps exp well within fp32 range).
  * LayerNorm scale/bias and the attention scale are folded into the
    weights on the host; v-bias and o_b fold into a constant row added on
    the host.  rstd = exp(-0.5*ln(var+eps)) and DVE-side bias adds keep
    the whole kernel on ONE ACT table set (natural_log_exp_and_others).
  * Scores use paired PSUM banks so each exp covers [128,1024].
  * Batch-level software pipeline: batch b+1's LayerNorm+transpose is
    emitted inside batch b's attention phase (right after the last q
    projection frees the xhT buffer slot) so the PE never waits on the
    serial LN chain at batch boundaries.
"""

import sys

import numpy as np

for _p in ("/opt/trn_rl_repo", "/root/.axon_site/_ro/trn_rl_repo"):
    if _p not in sys.path:
        sys.path.append(_p)

import concourse.bacc as bacc
import concourse.mybir as mybir
import concourse.tile as tile
from concourse.bass_utils import run_bass_kernel_spmd
from concourse.masks import make_identity

# Steer the ACT-table-load placement pass to the one set that holds every
# function this kernel uses (ln, exp, copy, identity), so the whole kernel
# runs on a single table load instead of thrashing between per-function
# sets.  Only the pass's view is doctored; runtime tables are untouched.
_ONE_SET = "natural_log_exp_and_others"
_orig_get_act_tables = bacc.get_activation_tables


def _patched_get_act_tables(arch):
    t = _orig_get_act_tables(arch)
    af = mybir.ActivationFunctionType
    strip = {af.Ln, af.Exp, af.Copy, af.Identity}
    return {
        name: (set(fns) if name == _ONE_SET else set(fns) - strip)
        for name, fns in t.items()
    }


bacc.get_activation_tables = _patched_get_act_tables

B, S, D, H = 4, 2048, 512, 8
P = 128
DC = D // P          # head/model dim chunks (4)
KC = S // P          # k chunks per batch (16)
QB = 512             # q-block size
NQB = S // QB        # q blocks per batch (4)
EPS = 1e-5
F32 = mybir.dt.float32
F32R = mybir.dt.float32r
AF = mybir.ActivationFunctionType
ALU = mybir.AluOpType

N_CORES = 8

_CACHE = {}


class _Kern:
    """Holds the pools/constants and emits the per-batch phases."""

    def __init__(self, nc, tc, pools):
        self.nc = nc
        self.tc = tc
        for k, v in pools.items():
            setattr(self, k, v)

    def setup_consts(self, qb_d, kb_d, w_drams):
        nc = self.nc
        self.ident = self.const.tile([P, P], F32, name="ident")
        make_identity(nc, self.ident)
        ones_raw = self.const.tile([P, 1], F32, name="ones_raw")
        nc.vector.memset(ones_raw, 1.0)
        self.ones_r = self.const.tile([P, 1], F32R, name="ones_r")
        nc.vector.tensor_copy(self.ones_r, ones_raw.bitcast(F32R))
        self.eps_t = self.const.tile([P, 1], F32, name="eps_t")
        nc.vector.memset(self.eps_t, EPS)
        self.qb_t = self.const.tile([P, DC], F32, name="qb_t")
        nc.gpsimd.dma_start(out=self.qb_t,
                            in_=qb_d.rearrange("(c p) -> p c", p=P))
        self.kb_t = self.const.tile([P, DC], F32, name="kb_t")
        nc.gpsimd.dma_start(out=self.kb_t,
                            in_=kb_d.rearrange("(c p) -> p c", p=P))
        # weights: load (SWDGE queue, so x loads aren't stuck behind) and
        # round to f32r
        self.w_r = {}
        for n, dram in w_drams.items():
            wst = self.big.tile([P, DC, D], F32, name=f"{n}_stage", tag="big")
            nc.gpsimd.dma_start(out=wst,
                                in_=dram.rearrange("(c p) n -> p c n", p=P))
            self.w_r[n] = self.wts.tile([P, DC, D], F32R, name=f"{n}_r", tag=n)
            nc.scalar.copy(self.w_r[n], wst.bitcast(F32R))

    # ---- phase A: LayerNorm + transpose -> xhT [d, r] ----
    def phase_a(self, x, b):
        nc = self.nc
        xhT = self.big.tile([P, DC, S], F32R, name=f"xhT{b}", tag="big")
        for g in range(KC // 2):
            xg = self.stage.tile([P, 2, D], F32, name="xg", tag="xg", bufs=2)
            r0 = g * 2 * P
            nc.sync.dma_start(
                out=xg,
                in_=x[b, r0:r0 + 2 * P, :].rearrange("(j p) d -> p j d", p=P))
            mvs, rstds = [], []
            for j in range(2):
                st6 = self.stats.tile([P, 6], F32, name="st6", tag=f"st6{j}")
                nc.vector.bn_stats(out=st6, in_=xg[:, j, :])
                mv = self.stats.tile([P, 2], F32, name="mv", tag=f"mv{j}")
                nc.vector.bn_aggr(out=mv, in_=st6)
                mvs.append(mv)
            for j in range(2):
                # rstd = exp(-0.5 * ln(var + eps))
                lnv = self.stats.tile([P, 1], F32, name="lnv", tag=f"lnv{j}")
                nc.scalar.activation(out=lnv, in_=mvs[j][:, 1:2], func=AF.Ln,
                                     bias=self.eps_t)
                rstd = self.stats.tile([P, 1], F32, name="rstd", tag=f"rstd{j}")
                nc.scalar.activation(out=rstd, in_=lnv, func=AF.Exp,
                                     scale=-0.5)
                rstds.append(rstd)
            for j in range(2):
                rt = g * 2 + j
                xh = self.stage.tile([P, D], F32, name="xh", tag="xh", bufs=2)
                nc.vector.tensor_scalar(out=xh, in0=xg[:, j, :],
                                        scalar1=mvs[j][:, 0:1],
                                        scalar2=rstds[j],
                                        op0=ALU.subtract, op1=ALU.mult)
                tp = self.psum.tile([P, D], F32, name="tp", tag="s", bufs=2)
                for dc in range(DC):
                    nc.tensor.transpose(tp[:, dc * P:(dc + 1) * P],
                                        xh[:, dc * P:(dc + 1) * P], self.ident)
                nc.scalar.copy(
                    out=xhT[:, :, rt * P:(rt + 1) * P],
                    in_=tp.rearrange("p (c r) -> p c r", c=DC).bitcast(F32R))
        return xhT

    # ---- phase B: k^T and v projections (full batch) ----
    def phase_b(self, xhT, b):
        nc = self.nc
        kT = self.kv.tile([P, DC, S], F32R, name=f"kT{b}", tag="kT")
        for cc in range(DC):
            for hf in range(2):
                kps = self.psum.tile([P, 2, QB], F32, name="kps", tag="s",
                                     bufs=2)
                for dc in range(DC):
                    for j in range(2):
                        q0 = (hf * 2 + j) * QB
                        nc.tensor.matmul(
                            kps[:, j, :],
                            self.w_r["kw"][:, dc, cc * P:(cc + 1) * P],
                            xhT[:, dc, q0:q0 + QB],
                            start=(dc == 0), stop=(dc == DC - 1))
                nc.vector.tensor_scalar_add(
                    out=kT[:, cc, hf * 2 * QB:(hf + 1) * 2 * QB],
                    in0=kps.rearrange("p j q -> p (j q)").bitcast(F32R),
                    scalar1=self.kb_t[:, cc:cc + 1])
        v_t = self.kv.tile([P, KC, D], F32R, name=f"v{b}", tag="v")
        for rp in range(KC // 2):
            vps = self.psum.tile([P, 2, D], F32, name="vps", tag="s", bufs=2)
            for dc in range(DC):
                for j in range(2):
                    rc = rp * 2 + j
                    nc.tensor.matmul(
                        vps[:, j, :], xhT[:, dc, rc * P:(rc + 1) * P],
                        self.w_r["vw"][:, dc, :],
                        start=(dc == 0), stop=(dc == DC - 1))
            nc.vector.tensor_copy(out=v_t[:, rp * 2:rp * 2 + 2, :],
                                  in_=vps.bitcast(F32R))
        return kT, v_t

    # ---- q^T projection for one q-block ----
    def qproj(self, xhT, qb_i):
        nc = self.nc
        q0 = qb_i * QB
        qT = self.qtp.tile([P, DC, QB], F32R, name=f"qT{qb_i}", tag="qT")
        for cp in range(DC // 2):
            qps = self.psum.tile([P, 2, QB], F32, name="qps", tag="s", bufs=2)
            for dc in range(DC):
                for j in range(2):
                    cc = cp * 2 + j
                    nc.tensor.matmul(
                        qps[:, j, :],
                        self.w_r["qw"][:, dc, cc * P:(cc + 1) * P],
                        xhT[:, dc, q0:q0 + QB],
                        start=(dc == 0), stop=(dc == DC - 1))
            for j in range(2):
                cc = cp * 2 + j
                nc.vector.tensor_scalar_add(out=qT[:, cc, :],
                                            in0=qps[:, j, :].bitcast(F32R),
                                            scalar1=self.qb_t[:, cc:cc + 1])
        return qT

    # ---- attention scores: S^T + exp for one q-block ----
    def attn_scores(self, qT, kT):
        nc = self.nc
        pT = self.big.tile([P, KC, QB], F32R, name="pT", tag="big")
        for kp in range(KC // 2):
            sps = self.psum.tile([P, 2, QB], F32, name="sps", tag="s", bufs=2)
            for dc in range(DC):
                for j in range(2):
                    kc = kp * 2 + j
                    nc.tensor.matmul(
                        sps[:, j, :], kT[:, dc, kc * P:(kc + 1) * P],
                        qT[:, dc, :],
                        start=(dc == 0), stop=(dc == DC - 1))
            nc.scalar.activation(out=pT[:, kp * 2:kp * 2 + 2, :],
                                 in_=sps.bitcast(F32R), func=AF.Exp)
        return pT

    # ---- attention l + att@V for one q-block ----
    def attn_av(self, lsum, pT, v_t, b, qb_i, skip_av=False):
        nc = self.nc
        q0 = qb_i * QB
        l_ps = self.psum.tile([1, QB], F32, name="l_ps", tag="ly", bufs=2)
        for kc in range(KC):
            nc.tensor.matmul(l_ps, self.ones_r, pT[:, kc, :],
                             start=(kc == 0), stop=(kc == KC - 1))
        l_sb = self.lsbp.tile([1, QB], F32, name="l_sb", tag="l")
        nc.vector.tensor_copy(out=l_sb, in_=l_ps)
        nc.sync.dma_start(out=lsum[b, q0:q0 + QB].unsqueeze(0), in_=l_sb)
        if skip_av:
            return None
        oT = self.otp.tile([P, DC, QB], F32R, name="oT", tag="oT")
        for dc in range(DC):
            o_ps = self.psum.tile([P, QB], F32, name="o_ps", tag="o", bufs=2)
            for kc in range(KC):
                nc.tensor.matmul(o_ps, v_t[:, kc, dc * P:(dc + 1) * P],
                                 pT[:, kc, :],
                                 start=(kc == 0), stop=(kc == KC - 1))
            nc.scalar.copy(out=oT[:, dc, :], in_=o_ps.bitcast(F32R))
        return oT

    # ---- attention tail: output projection + store ----
    def attn_tail(self, y, oT, b, qb_i):
        nc = self.nc
        q0 = qb_i * QB
        for qc in range(QB // P):
            yps = self.psum.tile([P, D], F32, name="yps", tag="ly", bufs=2)
            for dc in range(DC):
                nc.tensor.matmul(yps, oT[:, dc, qc * P:(qc + 1) * P],
                                 self.w_r["ow"][:, dc, :],
                                 start=(dc == 0), stop=(dc == DC - 1))
            yt = self.stage.tile([P, D], F32, name="yt", tag="yt", bufs=3)
            nc.vector.tensor_copy(out=yt, in_=yps)
            r0 = q0 + qc * P
            nc.sync.dma_start(out=y[b, r0:r0 + P, :], in_=yt)


def build(repeat=None, phases="full"):
    """repeat=R wraps the whole compute in a hardware For_i loop that runs
    it R times -- used only for wall-clock device-time benchmarking.
    phases in {"A", "AB", "ABS", "full"} truncates the pipeline (bench)."""
    import contextlib

    nc = bacc.Bacc("TRN2", target_bir_lowering=False, debug=False,
                   num_devices=N_CORES)
    x = nc.dram_tensor("x", [B, S, D], F32, kind="ExternalInput").ap()
    w_drams = {
        n: nc.dram_tensor(n, [D, D], F32, kind="ExternalInput").ap()
        for n in ("qw", "kw", "vw", "ow")
    }
    qb_d = nc.dram_tensor("qb", [D], F32, kind="ExternalInput").ap()
    kb_d = nc.dram_tensor("kb", [D], F32, kind="ExternalInput").ap()
    y = nc.dram_tensor("y", [B, S, D], F32, kind="ExternalOutput").ap()
    lsum = nc.dram_tensor("lsum", [B, S], F32, kind="ExternalOutput").ap()

    with tile.TileContext(nc) as tc:
        with (
            tc.tile_pool(name="const", bufs=1) as const,
            tc.tile_pool(name="wts", bufs=1) as wts,
            tc.tile_pool(name="kv", bufs=1) as kv,
            tc.tile_pool(name="big", bufs=2) as big,
            tc.tile_pool(name="qt", bufs=2) as qtp,
            tc.tile_pool(name="ot", bufs=1) as otp,
            tc.tile_pool(name="stage", bufs=1) as stage,
            tc.tile_pool(name="stats", bufs=4) as stats,
            tc.tile_pool(name="lsb", bufs=1) as lsbp,
            tc.tile_pool(name="psum", bufs=1, space="PSUM") as psum,
        ):
            k = _Kern(nc, tc, dict(const=const, wts=wts, kv=kv, big=big,
                                   qtp=qtp, otp=otp, stage=stage, stats=stats,
                                   lsbp=lsbp, psum=psum))
            k.setup_consts(qb_d, kb_d, w_drams)

            loop_cm = (tc.For_i(0, repeat, 1) if repeat
                       else contextlib.nullcontext())
            with loop_cm:
                xhT = k.phase_a(x, 0)
                for b in range(B):
                    nxt = None
                    if phases == "A":
                        if b + 1 < B:
                            nxt = k.phase_a(x, b + 1)
                        xhT = nxt
                        continue
                    kT, v_t = k.phase_b(xhT, b)
                    if phases == "AB":
                        if b + 1 < B:
                            nxt = k.phase_a(x, b + 1)
                        xhT = nxt
                        continue
                    skip_av = phases == "ABS"
                    qT0 = k.qproj(xhT, 0)
                    qT1 = k.qproj(xhT, 1)
                    pT0 = k.attn_scores(qT0, kT)
                    oT0 = k.attn_av(lsum, pT0, v_t, b, 0, skip_av)
                    if not skip_av:
                        k.attn_tail(y, oT0, b, 0)
                    qT2 = k.qproj(xhT, 2)
                    pT1 = k.attn_scores(qT1, kT)
                    qT3 = k.qproj(xhT, 3)
                    oT1 = k.attn_av(lsum, pT1, v_t, b, 1, skip_av)
                    # hoist next batch's LayerNorm into this batch's att@V
                    # window (xhT slot freed by qproj(3))
                    if b + 1 < B:
                        nxt = k.phase_a(x, b + 1)
                    if not skip_av:
                        k.attn_tail(y, oT1, b, 1)
                    pT2 = k.attn_scores(qT2, kT)
                    oT2 = k.attn_av(lsum, pT2, v_t, b, 2, skip_av)
                    if not skip_av:
                        k.attn_tail(y, oT2, b, 2)
                    pT3 = k.attn_scores(qT3, kT)
                    oT3 = k.attn_av(lsum, pT3, v_t, b, 3, skip_av)
                    if not skip_av:
                        k.attn_tail(y, oT3, b, 3)
                    xhT = nxt

    nc.compile()
    return nc


def _prep_core_inputs(inputs, h):
    """Fold LN affine + attention scale into per-head weights (float64)."""
    x = np.asarray(inputs["x"], np.float32)
    ln_w = np.asarray(inputs["ln_w"], np.float64)
    ln_b = np.asarray(inputs["ln_b"], np.float64)
    sl = slice(h * D, (h + 1) * D)
    scale = float(D) ** -0.5
    q_w = np.asarray(inputs["q_w"], np.float64)[:, sl]
    k_w = np.asarray(inputs["k_w"], np.float64)[:, sl]
    v_w = np.asarray(inputs["v_w"], np.float64)[:, sl]
    o_w = np.asarray(inputs["o_w"], np.float64)[sl, :]
    q_b = np.asarray(inputs["q_b"], np.float64)[sl]
    k_b = np.asarray(inputs["k_b"], np.float64)[sl]
    qw = (ln_w[:, None] * q_w) * scale
    kw = ln_w[:, None] * k_w
    vw = ln_w[:, None] * v_w
    qb = (ln_b @ q_w + q_b) * scale
    kb = ln_b @ k_w + k_b
    return {
        "x": x,
        "qw": qw.astype(np.float32), "kw": kw.astype(np.float32),
        "vw": vw.astype(np.float32), "ow": o_w.astype(np.float32),
        "qb": qb.astype(np.float32), "kb": kb.astype(np.float32),
    }


def kernel(**inputs):
    if "nc" not in _CACHE:
        _CACHE["nc"] = build()
    nc = _CACHE["nc"]

    in_maps = [_prep_core_inputs(inputs, h) for h in range(N_CORES)]
    res = run_bass_kernel_spmd(nc, in_maps, core_ids=list(range(N_CORES)))

    out = np.zeros((B, S, D), np.float64)
    for h in range(N_CORES):
        yh = res.results[h]["y"].astype(np.float64)
        lh = res.results[h]["lsum"].astype(np.float64)
        out += yh / lh[..., None]

    # host-folded constant row: sum_h vb_h @ ow_h + o_b
    ln_b = np.asarray(inputs["ln_b"], np.float64)
    v_w = np.asarray(inputs["v_w"], np.float64)
    v_b = np.asarray(inputs["v_b"], np.float64)
    o_w = np.asarray(inputs["o_w"], np.float64)
    o_b = np.asarray(inputs["o_b"], np.float64)
    vb_full = ln_b @ v_w + v_b            # [D*H]
    out += vb_full @ o_w + o_b
    return out.astype(np.float32)
